# revision 8
# baseline (speedup 1.0000x reference)
"""Trainium2 Bass kernel for nn_Decoder (dense MLP).

Computes out = relu(V @ W1 + b1) @ W2 + b2 for V [262144, 1024],
W1 [1024, 128], W2 [128, 4].

Strategy
--------
Data-parallel over 8 NeuronCores: V is sharded along rows (32768 rows per
core); the small weights are replicated. Each core's V shard is transposed
on the host to [1024, 32768] so the contraction dim (1024) lands on SBUF
partitions with fully contiguous DMA loads — no on-chip transposes.

Per core, the kernel computes h.T = W1.T @ V.T via PSUM-accumulated
matmuls over 8 K-chunks (lhsT = the natural W1 layout), applies
ReLU(+b1) on the scalar engine reading PSUM (emitting f16 h), then
out.T = W2.T @ h.T as a single f16 matmul, adds b2 on the vector
engine, and stores out.T [4, 32768] contiguously. The host transposes
the gathered outputs back.

The kernel is HBM-bound: V in f16 is 64 MiB/core against the ~358 GB/s
per-core HBM ceiling (~187 us). Everything else is engineered to keep
the 16 SDMA engines continuously busy: the first V chunk's DMA is
issued before anything else on the sync ring, weights load on the
scalar ring in parallel, and 4 group buffers of prefetch depth keep
the DMA queues fed. All matmuls run in f16 (1 col/cycle @ 2.4 GHz
warm) so the PE (~140 us) stays off the critical path.

Precision modes (KERNEL_MODE env var):
  f32    — plain fp32 matmuls (4x PE cycles, 2x DMA bytes).
  bf16   — single-pass bf16 (~3e-3 rel err).
  f16    — single-pass fp16 (~4e-4 rel err; default).
"""

import os
import sys

import numpy as np

for _p in ("/opt/trn_rl_repo", "/root/.axon_site/_ro/trn_rl_repo"):
    if os.path.isdir(_p) and _p not in sys.path:
        sys.path.insert(0, _p)

import concourse.bass as bass
import concourse.mybir as mybir
import concourse.tile as tile
from concourse import bacc
from concourse.bass_utils import run_bass_kernel_spmd

NCORES = 8
NN = 262144
IN_DIM = 1024
HIDDEN = 128
OUT_DIM = 4
R = NN // NCORES  # rows per core

P = 128           # SBUF partitions
KC = IN_DIM // P  # 8 k-chunks
CHUNK = 512       # rows per PSUM accumulation tile (one PSUM bank)
GROUP = 2048      # rows per DMA group
DATA_BUFS = 6     # prefetch depth for V-group tiles

MODE = os.environ.get("KERNEL_MODE", "f16")

_last_results = None  # exposed for test harness (exec_time_ns etc.)


def _moving_dtype(mode):
    if mode == "bf16":
        return mybir.dt.bfloat16
    if mode == "f16":
        return mybir.dt.float16
    return mybir.dt.float32


def build_nc(mode=MODE, rows=R):
    """Build the SPMD Bass program for one core."""
    f32 = mybir.dt.float32
    mdt = _moving_dtype(mode)

    nc = bacc.Bacc("TRN2")

    vth_d = nc.declare_dram_parameter("VTH", [IN_DIM, rows], mdt, isOutput=False)
    # W1 arrives host-prepacked in SBUF layout [P, KC*HIDDEN] so its DMA
    # moves 2 KB contiguous lines (128 descriptors) instead of 256 B ones.
    w1_d = nc.declare_dram_parameter("W1P", [P, KC * HIDDEN], mdt, isOutput=False)
    b1_d = nc.declare_dram_parameter("B1", [HIDDEN, 1], f32, isOutput=False)
    w2_d = nc.declare_dram_parameter("W2F", [HIDDEN, OUT_DIM], mdt, isOutput=False)
    b2_d = nc.declare_dram_parameter("B2", [OUT_DIM, 1], f32, isOutput=False)
    out_d = nc.declare_dram_parameter("OUT", [OUT_DIM, rows], f32, isOutput=True)

    ngroups = rows // GROUP
    nchunk = GROUP // CHUNK

    with tile.TileContext(nc) as tc:
        with (
            tc.tile_pool(name="const", bufs=1) as cpool,
            tc.tile_pool(name="data", bufs=DATA_BUFS) as dpool,
            tc.tile_pool(name="work", bufs=3) as wpool,
            tc.tile_pool(name="psum1", bufs=4, space="PSUM") as ppool,
            tc.tile_pool(name="psum2", bufs=2, space="PSUM") as opool,
        ):
            vth_view = vth_d[:].rearrange("(c p) (g n) -> g p c n", p=P, n=GROUP)
            out_view = out_d[:].rearrange("o (m n) -> m o n", n=CHUNK)

            # Bootstrap: put the first V chunk's DMA at the head of the
            # sync ring so the HBM stream starts immediately; weights
            # load on the scalar ring in parallel.
            vth0 = dpool.tile([P, KC, GROUP], mdt, tag="vth")
            nc.sync.dma_start(vth0[:, :, 0:CHUNK], vth_view[0][:, :, 0:CHUNK])

            w1_sb = cpool.tile([P, KC, HIDDEN], mdt)
            nc.scalar.dma_start(
                w1_sb[:], w1_d[:].rearrange("p (c h) -> p c h", c=KC)
            )
            b1_sb = cpool.tile([HIDDEN, 1], f32)
            nc.scalar.dma_start(b1_sb[:], b1_d[:])
            w2_sb = cpool.tile([HIDDEN, OUT_DIM], mdt)
            nc.scalar.dma_start(w2_sb[:], w2_d[:])
            b2_sb = cpool.tile([OUT_DIM, 1], f32)
            nc.scalar.dma_start(b2_sb[:], b2_d[:])

            nc.sync.dma_start(vth0[:, :, CHUNK:2 * CHUNK],
                              vth_view[0][:, :, CHUNK:2 * CHUNK])
            nc.sync.dma_start(vth0[:, :, 2 * CHUNK:], vth_view[0][:, :, 2 * CHUNK:])

            for g in range(ngroups):
                if g == 0:
                    vth = vth0
                else:
                    vth = dpool.tile([P, KC, GROUP], mdt, tag="vth")
                    if g == ngroups - 1:
                        # Split the last group in two so the trailing
                        # compute overlaps the tail of the DMA stream.
                        # Two pieces (not four): more would exceed the 8
                        # DMAHW semaphore lanes and serialize the issues.
                        half = GROUP // 2
                        nc.sync.dma_start(vth[:, :, :half], vth_view[g][:, :, :half])
                        nc.sync.dma_start(vth[:, :, half:], vth_view[g][:, :, half:])
                    else:
                        nc.sync.dma_start(vth[:], vth_view[g])

                for u in range(nchunk):
                    sl = slice(u * CHUNK, (u + 1) * CHUNK)

                    ps = ppool.tile([HIDDEN, CHUNK], f32, tag="ps")
                    for c in range(KC):
                        nc.tensor.matmul(
                            ps[:], w1_sb[:, c, :], vth[:, c, sl],
                            start=(c == 0), stop=(c == KC - 1),
                        )

                    h_sb = wpool.tile([HIDDEN, CHUNK], mdt, tag="h")
                    nc.scalar.activation(
                        h_sb[:], ps[:],
                        mybir.ActivationFunctionType.Relu,
                        bias=b1_sb[:],
                    )

                    po = opool.tile([OUT_DIM, CHUNK], f32, tag="po")
                    nc.tensor.matmul(po[:], w2_sb[:], h_sb[:], start=True, stop=True)

                    o_sb = wpool.tile([OUT_DIM, CHUNK], f32, tag="o")
                    nc.vector.tensor_scalar_add(o_sb[:], po[:], b2_sb[:])

                    nc.scalar.dma_start(out_view[g * nchunk + u], o_sb[:])

    return nc


def kernel(V, W1, b1, W2, b2):
    global _last_results
    mode = MODE
    mdt = _moving_dtype(mode)
    np_dt = {
        mybir.dt.float32: np.float32,
        mybir.dt.bfloat16: None,  # filled below (ml_dtypes)
        mybir.dt.float16: np.float16,
    }[mdt]
    if np_dt is None:
        import ml_dtypes

        np_dt = ml_dtypes.bfloat16

    V = np.asarray(V, dtype=np.float32)
    W1 = np.asarray(W1, dtype=np.float32)
    b1 = np.asarray(b1, dtype=np.float32)
    W2 = np.asarray(W2, dtype=np.float32)
    b2 = np.asarray(b2, dtype=np.float32)

    # Prepack W1 into the SBUF tile layout [P, KC*HIDDEN]:
    # element (c*P + p, h) of W1 lands at [p, c*HIDDEN + h].
    w1p = np.ascontiguousarray(
        W1.astype(np_dt).reshape(KC, P, HIDDEN).transpose(1, 0, 2).reshape(P, KC * HIDDEN)
    )
    common = {
        "W1P": w1p,
        "B1": np.ascontiguousarray(b1.reshape(HIDDEN, 1)),
        "W2F": np.ascontiguousarray(W2.astype(np_dt)),
        "B2": np.ascontiguousarray(b2.reshape(OUT_DIM, 1)),
    }

    in_maps = []
    for c in range(NCORES):
        shard = V[c * R : (c + 1) * R]  # [R, IN_DIM]
        m = {"VTH": np.ascontiguousarray(shard.T.astype(np_dt))}
        m.update(common)
        in_maps.append(m)

    nc = build_nc(mode, R)
    nc.finalize()
    res = run_bass_kernel_spmd(nc, in_maps, list(range(NCORES)))
    _last_results = res

    out = np.concatenate(
        [np.asarray(r["OUT"]).T for r in res.results], axis=0
    ).astype(np.float32)
    return out


# revision 11
# speedup vs baseline: 1.2454x; 1.2454x over previous
"""Trainium2 Bass kernel for nn_Decoder (dense MLP).

Computes out = relu(V @ W1 + b1) @ W2 + b2 for V [262144, 1024],
W1 [1024, 128], W2 [128, 4].

Strategy
--------
Data-parallel over 8 NeuronCores: V is sharded along rows (32768 rows per
core); the small weights are replicated. Each core's V shard is transposed
on the host to [1024, 32768] so the contraction dim (1024) lands on SBUF
partitions with fully contiguous DMA loads — no on-chip transposes.

Per core, the kernel computes h.T = W1.T @ V.T via PSUM-accumulated
matmuls over 8 K-chunks (lhsT = the natural W1 layout), applies
ReLU(+b1) on the scalar engine reading PSUM (emitting f16 h), then
out.T = W2.T @ h.T as a single f16 matmul, adds b2 on the vector
engine, and stores out.T [4, 32768] contiguously. The host transposes
the gathered outputs back.

The kernel is HBM-bound: V in f16 is 64 MiB/core against the ~358 GB/s
per-core HBM ceiling (~187 us). Everything else is engineered to keep
the 16 SDMA engines continuously busy: the first V chunk's DMA is
issued before anything else on the sync ring, weights load on the
scalar ring in parallel, and 4 group buffers of prefetch depth keep
the DMA queues fed. All matmuls run in f16 (1 col/cycle @ 2.4 GHz
warm) so the PE (~140 us) stays off the critical path.

Precision modes (KERNEL_MODE env var):
  f32    — plain fp32 matmuls (4x PE cycles, 2x DMA bytes).
  bf16   — single-pass bf16 (~3e-3 rel err).
  f16    — single-pass fp16 (~4e-4 rel err; default).
"""

import os
import sys

import numpy as np

for _p in ("/opt/trn_rl_repo", "/root/.axon_site/_ro/trn_rl_repo"):
    if os.path.isdir(_p) and _p not in sys.path:
        sys.path.insert(0, _p)

import concourse.bass as bass
import concourse.mybir as mybir
import concourse.tile as tile
from concourse import bacc
from concourse.bass_utils import run_bass_kernel_spmd

NCORES = 8
NN = 262144
IN_DIM = 1024
HIDDEN = 128
OUT_DIM = 4
R = NN // NCORES  # rows per core

P = 128           # SBUF partitions
KC = IN_DIM // P  # 8 k-chunks
CHUNK = 512       # rows per PSUM accumulation tile (one PSUM bank)
GROUP = 2048      # rows per DMA group
DATA_BUFS = 6     # prefetch depth for V-group tiles

MODE = os.environ.get("KERNEL_MODE", "f8")

_last_results = None  # exposed for test harness (exec_time_ns etc.)


def _moving_dtype(mode):
    """dtype V streams in. f8 = fp8 e3m4 (4 mantissa bits): halves the
    HBM traffic; the PE upconverts at full precision at 1 col/cycle
    (single-rate; double-pumping would truncate to e6m3). Weights and h
    stay f16 — mixed-dtype matmul is allowed for non-fp32 operands."""
    if mode == "bf16":
        return mybir.dt.bfloat16
    if mode == "f16":
        return mybir.dt.float16
    if mode == "f8":
        return mybir.dt.float8e3
    return mybir.dt.float32


def _weight_dtype(mode):
    if mode == "f8":
        return mybir.dt.float16
    return _moving_dtype(mode)


def build_nc(mode=MODE, rows=R):
    """Build the SPMD Bass program for one core."""
    f32 = mybir.dt.float32
    mdt = _moving_dtype(mode)
    wdt = _weight_dtype(mode)

    nc = bacc.Bacc("TRN2")

    vth_d = nc.declare_dram_parameter("VTH", [IN_DIM, rows], mdt, isOutput=False)
    # W1 arrives host-prepacked in SBUF layout [P, KC*HIDDEN] so its DMA
    # moves 2 KB contiguous lines (128 descriptors) instead of 256 B ones.
    w1_d = nc.declare_dram_parameter("W1P", [P, KC * HIDDEN], wdt, isOutput=False)
    b1_d = nc.declare_dram_parameter("B1", [HIDDEN, 1], f32, isOutput=False)
    w2_d = nc.declare_dram_parameter("W2F", [HIDDEN, OUT_DIM], wdt, isOutput=False)
    b2_d = nc.declare_dram_parameter("B2", [OUT_DIM, 1], f32, isOutput=False)
    out_d = nc.declare_dram_parameter("OUT", [OUT_DIM, rows], f32, isOutput=True)

    ngroups = rows // GROUP
    nchunk = GROUP // CHUNK

    with tile.TileContext(nc) as tc:
        with (
            tc.tile_pool(name="const", bufs=1) as cpool,
            tc.tile_pool(name="data", bufs=DATA_BUFS) as dpool,
            tc.tile_pool(name="work", bufs=3) as wpool,
            tc.tile_pool(name="psum1", bufs=4, space="PSUM") as ppool,
            tc.tile_pool(name="psum2", bufs=2, space="PSUM") as opool,
        ):
            vth_view = vth_d[:].rearrange("(c p) (g n) -> g p c n", p=P, n=GROUP)
            out_view = out_d[:].rearrange("o (m n) -> m o n", n=CHUNK)

            # Bootstrap: put the first V chunk's DMA at the head of the
            # sync ring so the HBM stream starts immediately; weights
            # load on the scalar ring in parallel.
            vth0 = dpool.tile([P, KC, GROUP], mdt, tag="vth")
            nc.sync.dma_start(vth0[:, :, 0:CHUNK], vth_view[0][:, :, 0:CHUNK])

            w1_sb = cpool.tile([P, KC, HIDDEN], wdt)
            nc.scalar.dma_start(
                w1_sb[:], w1_d[:].rearrange("p (c h) -> p c h", c=KC)
            )
            b1_sb = cpool.tile([HIDDEN, 1], f32)
            nc.scalar.dma_start(b1_sb[:], b1_d[:])
            w2_sb = cpool.tile([HIDDEN, OUT_DIM], wdt)
            nc.scalar.dma_start(w2_sb[:], w2_d[:])
            b2_sb = cpool.tile([OUT_DIM, 1], f32)
            nc.scalar.dma_start(b2_sb[:], b2_d[:])

            nc.sync.dma_start(vth0[:, :, CHUNK:2 * CHUNK],
                              vth_view[0][:, :, CHUNK:2 * CHUNK])
            nc.sync.dma_start(vth0[:, :, 2 * CHUNK:], vth_view[0][:, :, 2 * CHUNK:])

            for g in range(ngroups):
                if g == 0:
                    vth = vth0
                else:
                    vth = dpool.tile([P, KC, GROUP], mdt, tag="vth")
                    if g == ngroups - 1:
                        # Split the last group per chunk so the trailing
                        # compute overlaps the tail of the DMA stream.
                        for u in range(nchunk):
                            slu = slice(u * CHUNK, (u + 1) * CHUNK)
                            nc.sync.dma_start(vth[:, :, slu], vth_view[g][:, :, slu])
                    else:
                        nc.sync.dma_start(vth[:], vth_view[g])

                for u in range(nchunk):
                    sl = slice(u * CHUNK, (u + 1) * CHUNK)

                    ps = ppool.tile([HIDDEN, CHUNK], f32, tag="ps")
                    for c in range(KC):
                        nc.tensor.matmul(
                            ps[:], w1_sb[:, c, :], vth[:, c, sl],
                            start=(c == 0), stop=(c == KC - 1),
                        )

                    h_sb = wpool.tile([HIDDEN, CHUNK], wdt, tag="h")
                    nc.scalar.activation(
                        h_sb[:], ps[:],
                        mybir.ActivationFunctionType.Relu,
                        bias=b1_sb[:],
                    )

                    po = opool.tile([OUT_DIM, CHUNK], f32, tag="po")
                    nc.tensor.matmul(po[:], w2_sb[:], h_sb[:], start=True, stop=True)

                    o_sb = wpool.tile([OUT_DIM, CHUNK], f32, tag="o")
                    nc.vector.tensor_scalar_add(o_sb[:], po[:], b2_sb[:])

                    nc.scalar.dma_start(out_view[g * nchunk + u], o_sb[:])

    return nc


def kernel(V, W1, b1, W2, b2):
    global _last_results
    mode = MODE
    mdt = _moving_dtype(mode)

    def _np_dt(bdt):
        if bdt == mybir.dt.float32:
            return np.float32
        if bdt == mybir.dt.float16:
            return np.float16
        import ml_dtypes

        if bdt == mybir.dt.bfloat16:
            return ml_dtypes.bfloat16
        return ml_dtypes.float8_e3m4

    np_vdt = _np_dt(mdt)
    np_wdt = _np_dt(_weight_dtype(mode))

    V = np.asarray(V, dtype=np.float32)
    W1 = np.asarray(W1, dtype=np.float32)
    b1 = np.asarray(b1, dtype=np.float32)
    W2 = np.asarray(W2, dtype=np.float32)
    b2 = np.asarray(b2, dtype=np.float32)

    # Prepack W1 into the SBUF tile layout [P, KC*HIDDEN]:
    # element (c*P + p, h) of W1 lands at [p, c*HIDDEN + h].
    w1p = np.ascontiguousarray(
        W1.astype(np_wdt).reshape(KC, P, HIDDEN).transpose(1, 0, 2).reshape(P, KC * HIDDEN)
    )
    common = {
        "W1P": w1p,
        "B1": np.ascontiguousarray(b1.reshape(HIDDEN, 1)),
        "W2F": np.ascontiguousarray(W2.astype(np_wdt)),
        "B2": np.ascontiguousarray(b2.reshape(OUT_DIM, 1)),
    }

    in_maps = []
    for c in range(NCORES):
        shard = V[c * R : (c + 1) * R]  # [R, IN_DIM]
        m = {"VTH": np.ascontiguousarray(shard.T.astype(np_vdt))}
        m.update(common)
        in_maps.append(m)

    nc = build_nc(mode, R)
    nc.finalize()
    res = run_bass_kernel_spmd(nc, in_maps, list(range(NCORES)))
    _last_results = res

    out = np.concatenate(
        [np.asarray(r["OUT"]).T for r in res.results], axis=0
    ).astype(np.float32)
    return out


# revision 14
# speedup vs baseline: 1.3760x; 1.1049x over previous
"""Trainium2 Bass kernel for nn_Decoder (dense MLP).

Computes out = relu(V @ W1 + b1) @ W2 + b2 for V [262144, 1024],
W1 [1024, 128], W2 [128, 4].

Strategy
--------
Data-parallel over 8 NeuronCores: V is sharded along rows (32768 rows per
core); the small weights are replicated. Each core's V shard is transposed
on the host to [1024, 32768] so the contraction dim (1024) lands on SBUF
partitions with fully contiguous DMA loads — no on-chip transposes.

Per core, the kernel computes h.T = W1.T @ V.T via PSUM-accumulated
matmuls over 8 K-chunks (lhsT = the natural W1 layout), applies
ReLU(+b1) on the scalar engine reading PSUM (emitting f16 h), then
out.T = W2.T @ h.T as a single f16 matmul, adds b2 on the vector
engine, and stores out.T [4, 32768] contiguously. The host transposes
the gathered outputs back.

V streams as fp8 e3m4 (4 mantissa bits — fp8 e4m3 fails the accuracy
gate, e3m4 passes with ~1.4e-2 vs the 2e-2 budget): 32 MiB/core of HBM
traffic against the ~358 GB/s per-core ceiling. The PE consumes the
e3m4 moving operand against f16 stationary weights at 1 col/cycle with
a full-precision upconvert (double-pumped fp8 would truncate to e6m3
and fail the gate — deliberately avoided). The DMA stream is kept
continuous: first V chunk's DMA issues before anything else on the
sync ring, weights load prepacked on the scalar ring in parallel,
6 group buffers of prefetch, and the last group splits per-chunk so
trailing compute overlaps the stream tail.

Precision modes (KERNEL_MODE env var):
  f32    — plain fp32 matmuls (4x PE cycles, 4x DMA bytes).
  bf16   — single-pass bf16 (~3e-3 rel err).
  f16    — single-pass fp16 (~4e-4 rel err).
  f8     — V in fp8 e3m4, weights/h in f16 (~1.4e-2 rel err; default).
"""

import os
import sys

import numpy as np

for _p in ("/opt/trn_rl_repo", "/root/.axon_site/_ro/trn_rl_repo"):
    if os.path.isdir(_p) and _p not in sys.path:
        sys.path.insert(0, _p)

import concourse.bass as bass
import concourse.mybir as mybir
import concourse.tile as tile
from concourse import bacc
from concourse.bass_utils import run_bass_kernel_spmd

NCORES = 8
NN = 262144
IN_DIM = 1024
HIDDEN = 128
OUT_DIM = 4
R = NN // NCORES  # rows per core

P = 128           # SBUF partitions
KC = IN_DIM // P  # 8 k-chunks
CHUNK = 512       # rows per PSUM accumulation tile (one PSUM bank)
GROUP = 2048      # rows per DMA group
DATA_BUFS = 6     # prefetch depth for V-group tiles

MODE = os.environ.get("KERNEL_MODE", "f8")

_last_results = None  # exposed for test harness (exec_time_ns etc.)


def _moving_dtype(mode):
    """dtype V streams in. f8 = fp8 e3m4 (4 mantissa bits): halves the
    HBM traffic; the PE upconverts at full precision at 1 col/cycle
    (single-rate; double-pumping would truncate to e6m3). Weights and h
    stay f16 — mixed-dtype matmul is allowed for non-fp32 operands."""
    if mode == "bf16":
        return mybir.dt.bfloat16
    if mode == "f16":
        return mybir.dt.float16
    if mode == "f8":
        return mybir.dt.float8e3
    return mybir.dt.float32


def _weight_dtype(mode):
    if mode == "f8":
        return mybir.dt.float16
    return _moving_dtype(mode)


def build_nc(mode=MODE, rows=R):
    """Build the SPMD Bass program for one core."""
    f32 = mybir.dt.float32
    mdt = _moving_dtype(mode)
    wdt = _weight_dtype(mode)

    nc = bacc.Bacc("TRN2")

    vth_d = nc.declare_dram_parameter("VTH", [IN_DIM, rows], mdt, isOutput=False)
    # W1 arrives host-prepacked in SBUF layout [P, KC*HIDDEN] so its DMA
    # moves 2 KB contiguous lines (128 descriptors) instead of 256 B ones.
    w1_d = nc.declare_dram_parameter("W1P", [P, KC * HIDDEN], wdt, isOutput=False)
    b1_d = nc.declare_dram_parameter("B1", [HIDDEN, 1], f32, isOutput=False)
    w2_d = nc.declare_dram_parameter("W2F", [HIDDEN, OUT_DIM], wdt, isOutput=False)
    b2_d = nc.declare_dram_parameter("B2", [OUT_DIM, 1], f32, isOutput=False)
    out_d = nc.declare_dram_parameter("OUT", [OUT_DIM, rows], f32, isOutput=True)

    ngroups = rows // GROUP
    nchunk = GROUP // CHUNK

    with tile.TileContext(nc) as tc:
        with (
            tc.tile_pool(name="const", bufs=1) as cpool,
            tc.tile_pool(name="data", bufs=DATA_BUFS) as dpool,
            tc.tile_pool(name="work", bufs=3) as wpool,
            tc.tile_pool(name="psum1", bufs=3, space="PSUM") as ppool,
            tc.tile_pool(name="psum2", bufs=2, space="PSUM") as opool,
        ):
            vth_view = vth_d[:].rearrange("(c p) (g n) -> g p c n", p=P, n=GROUP)
            out_view = out_d[:].rearrange("o (m n) -> m o n", n=CHUNK)

            # Bootstrap: put the first V chunk's DMA at the head of the
            # sync ring so the HBM stream starts immediately; weights
            # load on the scalar ring in parallel.
            vth0 = dpool.tile([P, KC, GROUP], mdt, tag="vth")
            nc.sync.dma_start(vth0[:, :, 0:CHUNK], vth_view[0][:, :, 0:CHUNK])

            w1_sb = cpool.tile([P, KC, HIDDEN], wdt)
            nc.scalar.dma_start(
                w1_sb[:], w1_d[:].rearrange("p (c h) -> p c h", c=KC)
            )
            b1_sb = cpool.tile([HIDDEN, 1], f32)
            nc.scalar.dma_start(b1_sb[:], b1_d[:])
            w2_sb = cpool.tile([HIDDEN, OUT_DIM], wdt)
            nc.scalar.dma_start(w2_sb[:], w2_d[:])
            b2_sb = cpool.tile([OUT_DIM, 1], f32)
            nc.scalar.dma_start(b2_sb[:], b2_d[:])

            nc.sync.dma_start(vth0[:, :, CHUNK:2 * CHUNK],
                              vth_view[0][:, :, CHUNK:2 * CHUNK])
            nc.sync.dma_start(vth0[:, :, 2 * CHUNK:], vth_view[0][:, :, 2 * CHUNK:])

            for g in range(ngroups):
                if g == 0:
                    vth = vth0
                else:
                    vth = dpool.tile([P, KC, GROUP], mdt, tag="vth")
                    if g == ngroups - 1:
                        # Split the last group per chunk so the trailing
                        # compute overlaps the tail of the DMA stream.
                        for u in range(nchunk):
                            slu = slice(u * CHUNK, (u + 1) * CHUNK)
                            nc.sync.dma_start(vth[:, :, slu], vth_view[g][:, :, slu])
                    else:
                        nc.sync.dma_start(vth[:], vth_view[g])

                # Process chunks in pairs with interleaved matmuls: each
                # W1 k-chunk weight load serves two matmuls, so LDWEIGHTS
                # hides under the second matmul instead of gating every one.
                for up in range(nchunk // 2):
                    u0 = 2 * up
                    sl0 = slice(u0 * CHUNK, (u0 + 1) * CHUNK)
                    sl1 = slice((u0 + 1) * CHUNK, (u0 + 2) * CHUNK)

                    ps0 = ppool.tile([HIDDEN, CHUNK], f32, tag="ps")
                    ps1 = ppool.tile([HIDDEN, CHUNK], f32, tag="ps2")
                    for c in range(KC):
                        nc.tensor.matmul(
                            ps0[:], w1_sb[:, c, :], vth[:, c, sl0],
                            start=(c == 0), stop=(c == KC - 1),
                        )
                        nc.tensor.matmul(
                            ps1[:], w1_sb[:, c, :], vth[:, c, sl1],
                            start=(c == 0), stop=(c == KC - 1),
                        )

                    for u, ps in ((u0, ps0), (u0 + 1, ps1)):
                        h_sb = wpool.tile([HIDDEN, CHUNK], wdt, tag="h")
                        nc.scalar.activation(
                            h_sb[:], ps[:],
                            mybir.ActivationFunctionType.Relu,
                            bias=b1_sb[:],
                        )

                        po = opool.tile([OUT_DIM, CHUNK], f32, tag="po")
                        nc.tensor.matmul(po[:], w2_sb[:], h_sb[:], start=True, stop=True)

                        o_sb = wpool.tile([OUT_DIM, CHUNK], f32, tag="o")
                        nc.vector.tensor_scalar_add(o_sb[:], po[:], b2_sb[:])

                        nc.scalar.dma_start(out_view[g * nchunk + u], o_sb[:])

    return nc


def kernel(V, W1, b1, W2, b2):
    global _last_results
    mode = MODE
    mdt = _moving_dtype(mode)

    def _np_dt(bdt):
        if bdt == mybir.dt.float32:
            return np.float32
        if bdt == mybir.dt.float16:
            return np.float16
        import ml_dtypes

        if bdt == mybir.dt.bfloat16:
            return ml_dtypes.bfloat16
        return ml_dtypes.float8_e3m4

    np_vdt = _np_dt(mdt)
    np_wdt = _np_dt(_weight_dtype(mode))

    V = np.asarray(V, dtype=np.float32)
    W1 = np.asarray(W1, dtype=np.float32)
    b1 = np.asarray(b1, dtype=np.float32)
    W2 = np.asarray(W2, dtype=np.float32)
    b2 = np.asarray(b2, dtype=np.float32)

    # Prepack W1 into the SBUF tile layout [P, KC*HIDDEN]:
    # element (c*P + p, h) of W1 lands at [p, c*HIDDEN + h].
    w1p = np.ascontiguousarray(
        W1.astype(np_wdt).reshape(KC, P, HIDDEN).transpose(1, 0, 2).reshape(P, KC * HIDDEN)
    )
    common = {
        "W1P": w1p,
        "B1": np.ascontiguousarray(b1.reshape(HIDDEN, 1)),
        "W2F": np.ascontiguousarray(W2.astype(np_wdt)),
        "B2": np.ascontiguousarray(b2.reshape(OUT_DIM, 1)),
    }

    in_maps = []
    for c in range(NCORES):
        shard = V[c * R : (c + 1) * R]  # [R, IN_DIM]
        m = {"VTH": np.ascontiguousarray(shard.T.astype(np_vdt))}
        m.update(common)
        in_maps.append(m)

    nc = build_nc(mode, R)
    nc.finalize()
    res = run_bass_kernel_spmd(nc, in_maps, list(range(NCORES)))
    _last_results = res

    out = np.concatenate(
        [np.asarray(r["OUT"]).T for r in res.results], axis=0
    ).astype(np.float32)
    return out
